# revision 11
# baseline (speedup 1.0000x reference)
"""Bass/Trainium2 kernel for nn_BipolarMorphological2D — power-mean rewrite.

Math: each branch is y = exp(max_p(L[s,p] + K[p,g])) with L = ln(max(±x, .1)).
Approximate the max-plus reduction with a log-sum-exp at sharpness k=384:
    max_p(L + K) ~= (1/k)·ln(Σ_p e^{k(L - mL_s)}·e^{kK}) + mL_s
where mL_s = max_p L[s,p] (exact row max, keeps the exponentials in range).
The inner sum is a plain matmul A @ Bm with A = e^{k(L-mL)} (in [0,1], bf16)
and Bm = e^{kK} (bf16-range), so the 75M-MAC reduction runs on the PE at
1 cyc/row instead of ~300us of DVE tensor ops. Approximation error only
appears when several patch entries tie within ~1/k of the max; measured
end-to-end rel-l2 vs the reference is ~4.5e-3 (gate 2e-2).

Sharding: data-parallel over batch, one image per core (B=8, 8 cores).

Per core pipeline:
  Two DVE tensor_scalar passes give max(±x, .1) (both on DVE: the Pool/GPSIMD
  tensor_scalar is a ~17us software op on Q7 and it also stalls concurrent
  DVE work), one ACT ln -> L = ln(max(±x, .1)) f16 in a [64, 1200] row tile.
  Three shifted DMA copies build a [128, 1040] replica tile per branch (row
  groups a = i*32 taps; group 3 dead), so each PE transpose of
  L4[:, 128q+j : +128] yields the three (i, j) taps for tap-column j ->
  patches [s, p] in PSUM f16 (double-buffered pool). Per 4-chunk half: DVE
  tensor_reduce (XY over live rows) -> mL, one batched DVE subtract -> D f16
  (dead rows pinned at -30), PE transposes D back into a second PSUM tile,
  and the batched ACT exp (scale=KP) is also the PSUM->SBUF mover, landing A
  bf16 directly in matmul [p, s] layout. 24 accumulating bf16 matmuls per
  branch produce S[s, g] (g = 2x64 outputs); the output side folds the
  mu = e^mL scale into the per-q ACT exp bias: y' = exp(ln(S)/KP + mL), so
  the combine is 4 Pool tensor_tensor ops (y'11-y'12) + (y'22-y'21) + bias.
  The result R [128s, 8q, 64o] is DMAed out as one contiguous dump (128
  descriptors); the host untangles s = (h2, w) / q -> (h, w).
  Weights Bm = exp(KP*K), bias, and the identity tiles load once outside the
  rep loop (setup, like the input image load). Activation-table loads are
  pre-placed so the exp/ln set loads exactly once.
"""

import numpy as np

B, C, H, W, O = 8, 32, 32, 32, 64
FH, FW = 3, 3
HO, WO = H - FH + 1, W - FW + 1    # 30, 30
P = FH * FW * C                    # 288
G = 2 * O                          # 128 = (kernel e1/e2) x (o)
SW = 1024                          # padded spatial s = 32*h + w
NQ = SW // 128                     # 8 s-chunks
SP = 1104                          # padded x row length (max read 64+1040)
TW = 1040                          # replica tile width (max col read 1025)
SHIFT = 0.1
KP = 384.0                         # power-mean sharpness
NCORES = 8

_CACHE = {}


def _build_program(reps=1, outer=1):
    key = ("nc", reps, outer)
    if key in _CACHE:
        return _CACHE[key]

    import concourse.mybir as mybir
    import concourse.tile as tile
    from concourse import bacc
    from concourse.masks import make_identity

    f32 = mybir.dt.float32
    f16 = mybir.dt.float16
    bf16 = mybir.dt.bfloat16
    Alu = mybir.AluOpType
    Act = mybir.ActivationFunctionType

    nc = bacc.Bacc()

    xp = nc.dram_tensor("xp", [C, SP], f32, kind="ExternalInput")
    kkp = nc.dram_tensor("kkp", [384, G], f32, kind="ExternalInput")
    biasd = nc.dram_tensor("biasd", [128, O], f32, kind="ExternalInput")
    # output: R dump [128, NQ, O]; partition p = 32*h2 + w, h = 4*q + h2
    yr = nc.dram_tensor("yr", [128, NQ, O], f32, kind="ExternalOutput")

    with tile.TileContext(nc) as tc:
        with tc.tile_pool(name="const", bufs=1) as cpool, \
             tc.tile_pool(name="pp", bufs=2, space="PSUM") as pp, \
             tc.tile_pool(name="pd", bufs=1, space="PSUM") as pd, \
             tc.tile_pool(name="ps", bufs=1, space="PSUM") as ps:

            # X replicated twice on partitions 0:64 so one DVE tensor_scalar
            # with a per-partition ±1 scalar computes both branch clamps
            X2 = cpool.tile([2 * C, SP], f32)
            for half in range(2):
                nc.sync.dma_start(X2[C * half:C * (half + 1), :], xp[:])
            sgn = cpool.tile([2 * C, 1], f32)
            nc.vector.memset(sgn[0:C, :], 1.0)
            nc.vector.memset(sgn[C:2 * C, :], -1.0)
            identF = cpool.tile([128, 128], f16)
            make_identity(nc, identF[:])
            identB = cpool.tile([128, 128], bf16)
            make_identity(nc, identB[:])

            # ---- constants (once): Bm = exp(KP * K), rows packed host-side
            # as (j-chunk, i-group, c); dead i=3 groups are -100 so their Bm
            # rows underflow to 0
            Kk = cpool.tile([128, 3, G], f32)
            nc.sync.dma_start(
                Kk[:], kkp[:].rearrange("(m p) g -> p m g", m=3))
            Bm = cpool.tile([128, 3, G], bf16)
            nc.scalar.activation(Bm[:], Kk[:], Act.Exp, scale=KP)
            biasb = cpool.tile([128, O], f32)
            nc.sync.dma_start(biasb[:], biasd[:])

            for _outer in range(outer):
                for _rep in range(reps):
                    pr = _rep % 2  # parity tag suffix: cross-rep double buffer

                    # ---- L = ln(max(±x, SHIFT)) f16; u on partitions 0:32,
                    # v on 32:64 so one ACT ln covers both branches
                    UVs = cpool.tile([64, SP], f32, tag=f"UVs{pr}")
                    LUV = cpool.tile([64, SP], f16, tag=f"LUV{pr}")
                    nc.vector.tensor_scalar(
                        out=UVs[:], in0=X2[:], scalar1=sgn[:], scalar2=SHIFT,
                        op0=Alu.mult, op1=Alu.max)
                    nc.scalar.activation(LUV[:], UVs[:], Act.Ln)

                    # ---- shift-baked replicas: L4[32a+c, t] = L[c, t+32a]
                    # (row-offset groups a=i in {0,1,2}; group 3 is dead and
                    # pinned at -10 so it never wins the row max. A chunk-q
                    # transpose of L4[:, 128q+j : +128] then yields all three
                    # (i, j) taps for that j in one [128, 128] block.)
                    L4 = {}
                    for bi, nm in enumerate(("u", "v")):
                        Lsrc = LUV[C * bi:C * (bi + 1), :]
                        t4 = cpool.tile([128, TW], f16, tag=f"L4{nm}{pr}")
                        if _rep < 2 and _outer == 0:
                            nc.vector.memset(t4[96:128, :], -10.0)
                        for a in range(3):
                            nc.sync.dma_start(
                                t4[32 * a:32 * (a + 1), :],
                                Lsrc[:, 32 * a:32 * a + TW])
                        L4[nm] = t4

                    # ---- per branch: patches -> mL -> D -> D^T -> A=exp
                    AT = {}
                    mLuv = cpool.tile([128, 2, NQ], f32, tag=f"mL{pr}")
                    for bi, nm in enumerate(("u", "v")):
                        mLb = mLuv[:, bi, :]
                        Db = cpool.tile([128, NQ, 384], f16, tag=f"D{nm}{pr}")
                        At = cpool.tile([128, NQ, 3, 128], bf16, tag=f"At{nm}{pr}")
                        if _rep < 2 and _outer == 0:
                            # dead rows (a=3 of each j-block): defined, tiny A
                            nc.vector.memset(
                                Db[:, :, :].rearrange(
                                    "s q (pc e) -> s q pc e", pc=3)[
                                    :, :, :, 96:128], -30.0)
                        for h in range(2):
                            Pp = pp.tile([128, 4, 3, 128], f16)
                            for qq in range(4):
                                q = 4 * h + qq
                                for pc in range(3):
                                    nc.tensor.transpose(
                                        Pp[:, qq, pc, :],
                                        L4[nm][:, 128 * q + pc:
                                               128 * q + pc + 128],
                                        identF[:])
                            # live-rows view [s, q, pc, 96] skips dead rows
                            Pl = Pp[:, :, :, 0:96]
                            nc.vector.tensor_reduce(
                                out=mLb[:, 4 * h:4 * h + 4], in_=Pl,
                                axis=mybir.AxisListType.XY, op=Alu.max)
                            # per-q tensor_scalar with the per-partition mL as
                            # AP scalar: all-f16 operands let DVE bit-pack 2x
                            for qq in range(4):
                                nc.vector.tensor_scalar(
                                    out=Db[:, 4 * h + qq, :].rearrange(
                                        "s (pc e) -> s pc e", pc=3)[
                                        :, :, 0:96],
                                    in0=Pp[:, qq, :, 0:96],
                                    scalar1=mLb[:, 4 * h + qq:4 * h + qq + 1],
                                    scalar2=None, op0=Alu.subtract)
                            # D^T in its own PSUM tile so the patch tile is
                            # released after the sub; the exp pass is also the
                            # PSUM->SBUF mover
                            Dps = pd.tile([128, 4, 3, 128], f16)
                            for qq in range(4):
                                q = 4 * h + qq
                                for pc in range(3):
                                    nc.tensor.transpose(
                                        Dps[:, qq, pc, :],
                                        Db[:, q, 128 * pc:128 * (pc + 1)],
                                        identF[:])
                            nc.scalar.activation(
                                At[:, 4 * h:4 * h + 4, :, :], Dps[:],
                                Act.Exp, scale=KP)
                        AT[nm] = At

                    # ---- matmul S[s,g] accumulated over 3 p-chunks.
                    # y' = S^(1/KP) runs ln->exp back-to-back on ACT (no
                    # cross-engine hop); the mu = e^mL scale is applied to the
                    # branch differences inside the Pool combine.
                    Y = {}
                    Mu = cpool.tile([128, 2, NQ], f32, tag=f"Mu{pr}")
                    nc.scalar.activation(Mu[:], mLuv[:], Act.Exp)
                    for bi, nm in enumerate(("u", "v")):
                        At = AT[nm]
                        Sps = ps.tile([128, NQ, G], f32, tag="S")
                        for q in range(NQ):
                            for pc in range(3):
                                nc.tensor.matmul(
                                    Sps[:, q, :],
                                    At[:, q, pc, :],
                                    Bm[:, pc, :],
                                    start=(pc == 0), stop=(pc == 2))
                        lnS = cpool.tile([128, NQ, G], f16, tag=f"lnS{nm}{pr}")
                        nc.scalar.activation(lnS[:], Sps[:], Act.Ln)
                        Yb = cpool.tile([128, NQ, G], f32, tag=f"Y{nm}{pr}")
                        nc.scalar.activation(Yb[:], lnS[:], Act.Exp,
                                             scale=1.0 / KP)
                        Y[nm] = Yb

                    # ---- combine mu_u*(y11'-y12') - mu_v*(y21'-y22') + bias
                    R = cpool.tile([128, NQ, O], f32, tag=f"R{pr}")
                    Rv = cpool.tile([128, NQ, O], f32, tag=f"Rv{pr}")
                    nc.gpsimd.tensor_tensor(
                        out=R[:], in0=Y["u"][:, :, 0:O],
                        in1=Y["u"][:, :, O:G], op=Alu.subtract)
                    nc.gpsimd.tensor_tensor(
                        out=R[:], in0=R[:],
                        in1=Mu[:, 0, :].unsqueeze(2)
                            .broadcast_to((128, NQ, O)),
                        op=Alu.mult)
                    nc.gpsimd.tensor_tensor(
                        out=Rv[:], in0=Y["v"][:, :, O:G],
                        in1=Y["v"][:, :, 0:O], op=Alu.subtract)
                    nc.gpsimd.tensor_tensor(
                        out=Rv[:], in0=Rv[:],
                        in1=Mu[:, 1, :].unsqueeze(2)
                            .broadcast_to((128, NQ, O)),
                        op=Alu.mult)
                    nc.gpsimd.tensor_tensor(
                        out=R[:], in0=R[:], in1=Rv[:], op=Alu.add)
                    nc.gpsimd.tensor_tensor(
                        out=R[:], in0=R[:],
                        in1=biasb[:].unsqueeze(1).broadcast_to((128, NQ, O)),
                        op=Alu.add)

                    # ---- out: one contiguous dump, host untangles layout
                    nc.sync.dma_start(yr[:], R[:])

    # Pre-place activation table loads using a view of the tables where only
    # the combined natural_log_exp set provides Exp/Ln. Set ids stay aligned
    # with act_info.json (index order unchanged); this just stops the
    # first-match chooser from alternating between the exp-only and ln-only
    # sets (7 reloads x 1.28us -> 1 load). compile()'s own pass then sees
    # every activation covered and inserts nothing.
    import bass_rust as _bass_rust
    from concourse.hw_specs import get_activation_tables
    tabs = []
    for name, fns in get_activation_tables(nc.m.arch).items():
        if name != "natural_log_exp_and_others":
            fns = fns - {mybir.ActivationFunctionType.Exp,
                         mybir.ActivationFunctionType.Ln}
        tabs.append((name, fns))
    _bass_rust.insert_act_table_loads(nc, tabs)

    nc.compile()
    _CACHE[key] = nc
    return nc


def _get_runner(reps=1, outer=1):
    """Cached jitted SPMD executor (replicates bass2jax.run_bass_via_pjrt but
    reuses the jitted callable across calls so we don't re-trace every time)."""
    key = ("run", reps, outer)
    if key in _CACHE:
        return _CACHE[key]

    import jax
    from jax.sharding import Mesh, PartitionSpec
    try:
        from jax.experimental.shard_map import shard_map
    except ImportError:  # newer jax
        from jax.shard_map import shard_map
    from concourse import bass2jax, mybir

    nc = _build_program(reps, outer)
    bass2jax.install_neuronx_cc_hook()

    partition_name = nc.partition_id_tensor.name if nc.partition_id_tensor else None
    in_names, out_names, out_avals, zero_outs = [], [], [], []
    for alloc in nc.m.functions[0].allocations:
        if not isinstance(alloc, mybir.MemoryLocationSet):
            continue
        name = alloc.memorylocations[0].name
        if alloc.kind == "ExternalInput":
            if name != partition_name:
                in_names.append(name)
        elif alloc.kind == "ExternalOutput":
            shape = tuple(alloc.tensor_shape)
            dtype = mybir.dt.np(alloc.dtype)
            out_names.append(name)
            out_avals.append(jax.core.ShapedArray(shape, dtype))
            zero_outs.append(np.zeros(shape, dtype))
    n_params = len(in_names)
    n_outs = len(out_avals)
    all_in_names = list(in_names) + list(out_names)
    if partition_name is not None:
        all_in_names.append(partition_name)
    donate = tuple(range(n_params, n_params + n_outs))

    def _body(*args):
        operands = list(args)
        if partition_name is not None:
            operands.append(bass2jax.partition_id_tensor())
        outs = bass2jax._bass_exec_p.bind(
            *operands,
            out_avals=tuple(out_avals),
            in_names=tuple(all_in_names),
            out_names=tuple(out_names),
            lowering_input_output_aliases=(),
            sim_require_finite=True,
            sim_require_nnan=True,
            nc=nc,
        )
        return tuple(outs)

    devices = jax.devices()[:NCORES]
    mesh = Mesh(np.asarray(devices), ("core",))
    sharded = jax.jit(
        shard_map(_body, mesh=mesh,
                  in_specs=(PartitionSpec("core"),) * (n_params + n_outs),
                  out_specs=(PartitionSpec("core"),) * n_outs,
                  check_rep=False),
        donate_argnums=donate,
        keep_unused=True,
    )

    def run(in_maps):
        concat_in = [
            np.concatenate([np.asarray(m[name]) for m in in_maps], axis=0)
            for name in in_names
        ]
        concat_zeros = [
            np.zeros((NCORES * z.shape[0], *z.shape[1:]), z.dtype)
            for z in zero_outs
        ]
        out_arrs = sharded(*concat_in, *concat_zeros)
        return [
            {name: np.asarray(out_arrs[i]).reshape(NCORES, *out_avals[i].shape)[c]
             for i, name in enumerate(out_names)}
            for c in range(NCORES)
        ]

    _CACHE[key] = run
    return run


def _make_in_maps(x, k1, k2, bias):
    # host-side layout prep (sharding + padding + row packing only)
    kkf = np.concatenate(
        [k1.reshape(P, O), k2.reshape(P, O)], axis=1).astype(np.float32)
    kkp = np.full((384, G), -100.0, dtype=np.float32)
    for j in range(3):
        for a in range(3):
            srow = (FW * a + j) * C
            kkp[128 * j + 32 * a:128 * j + 32 * (a + 1)] = kkf[srow:srow + C]
    biasd = np.ascontiguousarray(
        np.tile(bias[None, :], (128, 1)).astype(np.float32))
    in_maps = []
    for b in range(NCORES):
        xp = np.full((C, SP), SHIFT, dtype=np.float32)
        xp[:, :H * W] = x[b].reshape(C, H * W)
        in_maps.append({"xp": xp, "kkp": kkp, "biasd": biasd})
    return in_maps


def kernel(x, k1, k2, bias, reps=1, outer=1):
    x = np.asarray(x, dtype=np.float32)
    k1 = np.asarray(k1, dtype=np.float32)
    k2 = np.asarray(k2, dtype=np.float32)
    bias = np.asarray(bias, dtype=np.float32)

    run = _get_runner(reps, outer)
    results = run(_make_in_maps(x, k1, k2, bias))
    out = np.empty((B, O, HO, WO), dtype=np.float32)
    for b in range(NCORES):
        yrb = results[b]["yr"]  # [128, NQ, O]; partition = 32*h2 + w
        full = yrb.reshape(4, 32, NQ, O).transpose(3, 2, 0, 1).reshape(
            O, 4 * NQ, 32)  # [o, h = 4q + h2, w]
        out[b] = full[:, :HO, :WO]
    return out


# revision 14
# speedup vs baseline: 1.0238x; 1.0238x over previous
"""Bass/Trainium2 kernel for nn_BipolarMorphological2D — power-mean rewrite.

Math: each branch is y = exp(max_p(L[s,p] + K[p,g])) with L = ln(max(±x, .1)).
Approximate the max-plus reduction with a log-sum-exp at sharpness k=384:
    max_p(L + K) ~= (1/k)·ln(Σ_p e^{k(L - mL_s)}·e^{kK}) + mL_s
where mL_s = max_p L[s,p] (exact row max, keeps the exponentials in range).
The inner sum is a plain matmul A @ Bm with A = e^{k(L-mL)} (in [0,1], bf16)
and Bm = e^{kK} (bf16-range), so the 75M-MAC reduction runs on the PE at
1 cyc/row instead of ~300us of DVE tensor ops. Approximation error only
appears when several patch entries tie within ~1/k of the max; measured
end-to-end rel-l2 vs the reference is ~4.5e-3 (gate 2e-2).

Sharding: data-parallel over batch, one image per core (B=8, 8 cores).

Per core pipeline:
  Two DVE tensor_scalar passes give max(±x, .1) (both on DVE: the Pool/GPSIMD
  tensor_scalar is a ~17us software op on Q7 and it also stalls concurrent
  DVE work), one ACT ln -> L = ln(max(±x, .1)) f16 in a [64, 1200] row tile.
  Three shifted DMA copies build a [128, 1040] replica tile per branch (row
  groups a = i*32 taps; group 3 dead), so each PE transpose of
  L4[:, 128q+j : +128] yields the three (i, j) taps for tap-column j ->
  patches [s, p] in PSUM f16 (double-buffered pool). Per 4-chunk half: DVE
  tensor_reduce (XY over live rows) -> mL, one batched DVE subtract -> D f16
  (dead rows pinned at -30), PE transposes D back into a second PSUM tile,
  and the batched ACT exp (scale=KP) is also the PSUM->SBUF mover, landing A
  bf16 directly in matmul [p, s] layout. 24 accumulating bf16 matmuls per
  branch produce S[s, g] (g = 2x64 outputs); the output side folds the
  mu = e^mL scale into the per-q ACT exp bias: y' = exp(ln(S)/KP + mL), so
  the combine is 4 Pool tensor_tensor ops (y'11-y'12) + (y'22-y'21) + bias.
  The result R [128s, 8q, 64o] is DMAed out as one contiguous dump (128
  descriptors); the host untangles s = (h2, w) / q -> (h, w).
  Weights Bm = exp(KP*K), bias, and the identity tiles load once outside the
  rep loop (setup, like the input image load). Activation-table loads are
  pre-placed so the exp/ln set loads exactly once.
"""

import numpy as np

B, C, H, W, O = 8, 32, 32, 32, 64
FH, FW = 3, 3
HO, WO = H - FH + 1, W - FW + 1    # 30, 30
P = FH * FW * C                    # 288
G = 2 * O                          # 128 = (kernel e1/e2) x (o)
SW = 1024                          # padded spatial s = 32*h + w
NQ = SW // 128                     # 8 s-chunks
SP = 1104                          # padded x row length (max read 64+1040)
TW = 1040                          # replica tile width (max col read 1025)
SHIFT = 0.1
KP = 384.0                         # power-mean sharpness
NPAR = 3                           # cross-rep buffer depth
NCORES = 8

_CACHE = {}


def _build_program(reps=1, outer=1):
    key = ("nc", reps, outer)
    if key in _CACHE:
        return _CACHE[key]

    import concourse.mybir as mybir
    import concourse.tile as tile
    from concourse import bacc
    from concourse.masks import make_identity

    f32 = mybir.dt.float32
    f16 = mybir.dt.float16
    bf16 = mybir.dt.bfloat16
    Alu = mybir.AluOpType
    Act = mybir.ActivationFunctionType

    nc = bacc.Bacc()

    xp = nc.dram_tensor("xp", [C, SP], f32, kind="ExternalInput")
    kkp = nc.dram_tensor("kkp", [384, G], f32, kind="ExternalInput")
    biasd = nc.dram_tensor("biasd", [128, O], f32, kind="ExternalInput")
    # output: R dump [128, NQ, O]; partition p = 32*h2 + w, h = 4*q + h2
    yr = nc.dram_tensor("yr", [128, NQ, O], f32, kind="ExternalOutput")

    with tile.TileContext(nc) as tc:
        with tc.tile_pool(name="const", bufs=1) as cpool, \
             tc.tile_pool(name="pp", bufs=2, space="PSUM") as pp, \
             tc.tile_pool(name="pd", bufs=1, space="PSUM") as pd, \
             tc.tile_pool(name="ps", bufs=1, space="PSUM") as ps:

            # X replicated twice on partitions 0:64 so one DVE tensor_scalar
            # with a per-partition ±1 scalar computes both branch clamps
            X2 = cpool.tile([2 * C, SP], f32)
            for half in range(2):
                nc.sync.dma_start(X2[C * half:C * (half + 1), :], xp[:])
            sgn = cpool.tile([2 * C, 1], f32)
            nc.vector.memset(sgn[0:C, :], 1.0)
            nc.vector.memset(sgn[C:2 * C, :], -1.0)
            identF = cpool.tile([128, 128], f16)
            make_identity(nc, identF[:])
            identB = cpool.tile([128, 128], bf16)
            make_identity(nc, identB[:])

            # ---- constants (once): Bm = exp(KP * K), rows packed host-side
            # as (j-chunk, i-group, c); dead i=3 groups are -100 so their Bm
            # rows underflow to 0
            Kk = cpool.tile([128, 3, G], f32)
            nc.sync.dma_start(
                Kk[:], kkp[:].rearrange("(m p) g -> p m g", m=3))
            Bm = cpool.tile([128, 3, G], bf16)
            nc.scalar.activation(Bm[:], Kk[:], Act.Exp, scale=KP)
            biasb = cpool.tile([128, O], f32)
            nc.sync.dma_start(biasb[:], biasd[:])

            for _outer in range(outer):
                for _rep in range(reps):
                    pr = _rep % NPAR  # parity tag suffix: cross-rep buffering

                    # ---- L = ln(max(±x, SHIFT)) f16; u on partitions 0:32,
                    # v on 32:64 so one ACT ln covers both branches
                    UVs = cpool.tile([64, SP], f32, tag=f"UVs{pr}")
                    LUV = cpool.tile([64, SP], f16, tag=f"LUV{pr}")
                    nc.vector.tensor_scalar(
                        out=UVs[:], in0=X2[:], scalar1=sgn[:], scalar2=SHIFT,
                        op0=Alu.mult, op1=Alu.max)
                    nc.scalar.activation(LUV[:], UVs[:], Act.Ln)

                    # ---- shift-baked replicas: L4[32a+c, t] = L[c, t+32a]
                    # (row-offset groups a=i in {0,1,2}; group 3 is dead and
                    # pinned at -10 so it never wins the row max. A chunk-q
                    # transpose of L4[:, 128q+j : +128] then yields all three
                    # (i, j) taps for that j in one [128, 128] block.)
                    L4 = {}
                    for bi, nm in enumerate(("u", "v")):
                        Lsrc = LUV[C * bi:C * (bi + 1), :]
                        t4 = cpool.tile([128, TW], f16, tag=f"L4{nm}{pr}")
                        if _rep < NPAR and _outer == 0:
                            nc.vector.memset(t4[96:128, :], -10.0)
                        for a in range(3):
                            nc.sync.dma_start(
                                t4[32 * a:32 * (a + 1), :],
                                Lsrc[:, 32 * a:32 * a + TW])
                        L4[nm] = t4

                    # ---- per branch: patches -> mL -> D -> D^T -> A=exp
                    AT = {}
                    mLuv = cpool.tile([128, 2, NQ], f32, tag=f"mL{pr}")
                    for bi, nm in enumerate(("u", "v")):
                        mLb = mLuv[:, bi, :]
                        Db = cpool.tile([128, NQ, 384], f16, tag=f"D{nm}{pr}")
                        At = cpool.tile([128, NQ, 3, 128], bf16, tag=f"At{nm}{pr}")
                        if _rep < NPAR and _outer == 0:
                            # dead rows (a=3 of each j-block): defined, tiny A
                            nc.vector.memset(
                                Db[:, :, :].rearrange(
                                    "s q (pc e) -> s q pc e", pc=3)[
                                    :, :, :, 96:128], -30.0)
                            # w-pad columns (w=30,31) skipped by the exp: keep
                            # them defined for the matmul reads
                            nc.vector.memset(
                                At[:, :, :, :].rearrange(
                                    "p q pc (sh sw) -> p q pc sh sw", sw=32)[
                                    :, :, :, :, 30:32], 0.0)
                        for h in range(2):
                            Pp = pp.tile([128, 4, 3, 128], f16)
                            for qq in range(4):
                                q = 4 * h + qq
                                for pc in range(3):
                                    nc.tensor.transpose(
                                        Pp[:, qq, pc, :],
                                        L4[nm][:, 128 * q + pc:
                                               128 * q + pc + 128],
                                        identF[:])
                            # live-rows view [s, q, pc, 96] skips dead rows
                            Pl = Pp[:, :, :, 0:96]
                            nc.vector.tensor_reduce(
                                out=mLb[:, 4 * h:4 * h + 4], in_=Pl,
                                axis=mybir.AxisListType.XY, op=Alu.max)
                            # per-q tensor_scalar with the per-partition mL as
                            # AP scalar: all-f16 operands let DVE bit-pack 2x
                            for qq in range(4):
                                nc.vector.tensor_scalar(
                                    out=Db[:, 4 * h + qq, :].rearrange(
                                        "s (pc e) -> s pc e", pc=3)[
                                        :, :, 0:96],
                                    in0=Pp[:, qq, :, 0:96],
                                    scalar1=mLb[:, 4 * h + qq:4 * h + qq + 1],
                                    scalar2=None, op0=Alu.subtract)
                            # D^T in its own PSUM tile so the patch tile is
                            # released after the sub; the exp pass is also the
                            # PSUM->SBUF mover
                            Dps = pd.tile([128, 4, 3, 128], f16)
                            for qq in range(4):
                                q = 4 * h + qq
                                for pc in range(3):
                                    nc.tensor.transpose(
                                        Dps[:, qq, pc, :],
                                        Db[:, q, 128 * pc:128 * (pc + 1)],
                                        identF[:])
                            nc.scalar.activation(
                                At[:, 4 * h:4 * h + 4, :, :].rearrange(
                                    "p q pc (sh sw) -> p q pc sh sw", sw=32)[
                                    :, :, :, :, 0:30],
                                Dps[:].rearrange(
                                    "p q pc (sh sw) -> p q pc sh sw", sw=32)[
                                    :, :, :, :, 0:30],
                                Act.Exp, scale=KP)
                        AT[nm] = At

                    # ---- matmul S[s,g] accumulated over 3 p-chunks.
                    # y' = S^(1/KP) runs ln->exp back-to-back on ACT (no
                    # cross-engine hop); the mu = e^mL scale is applied to the
                    # branch differences inside the Pool combine.
                    Y = {}
                    Mu = cpool.tile([128, 2, NQ], f32, tag=f"Mu{pr}")
                    nc.scalar.activation(Mu[:], mLuv[:], Act.Exp)
                    for bi, nm in enumerate(("u", "v")):
                        At = AT[nm]
                        Sps = ps.tile([128, NQ, G], f32, tag="S")
                        for q in range(NQ):
                            for pc in range(3):
                                nc.tensor.matmul(
                                    Sps[:, q, :],
                                    At[:, q, pc, :],
                                    Bm[:, pc, :],
                                    start=(pc == 0), stop=(pc == 2))
                        lnS = cpool.tile([128, NQ, G], f16, tag=f"lnS{nm}{pr}")
                        nc.scalar.activation(lnS[:], Sps[:], Act.Ln)
                        Yb = cpool.tile([128, NQ, G], f32, tag=f"Y{nm}{pr}")
                        nc.scalar.activation(Yb[:], lnS[:], Act.Exp,
                                             scale=1.0 / KP)
                        Y[nm] = Yb

                    # ---- combine mu_u*(y11'-y12') - mu_v*(y21'-y22') + bias
                    R = cpool.tile([128, NQ, O], f32, tag=f"R{pr}")
                    Rv = cpool.tile([128, NQ, O], f32, tag=f"Rv{pr}")
                    nc.gpsimd.tensor_tensor(
                        out=R[:], in0=Y["u"][:, :, 0:O],
                        in1=Y["u"][:, :, O:G], op=Alu.subtract)
                    nc.gpsimd.tensor_tensor(
                        out=R[:], in0=R[:],
                        in1=Mu[:, 0, :].unsqueeze(2)
                            .broadcast_to((128, NQ, O)),
                        op=Alu.mult)
                    nc.gpsimd.tensor_tensor(
                        out=Rv[:], in0=Y["v"][:, :, O:G],
                        in1=Y["v"][:, :, 0:O], op=Alu.subtract)
                    nc.gpsimd.tensor_tensor(
                        out=Rv[:], in0=Rv[:],
                        in1=Mu[:, 1, :].unsqueeze(2)
                            .broadcast_to((128, NQ, O)),
                        op=Alu.mult)
                    nc.gpsimd.tensor_tensor(
                        out=R[:], in0=R[:], in1=Rv[:], op=Alu.add)
                    nc.gpsimd.tensor_tensor(
                        out=R[:], in0=R[:],
                        in1=biasb[:].unsqueeze(1).broadcast_to((128, NQ, O)),
                        op=Alu.add)

                    # ---- out: one contiguous dump, host untangles layout
                    nc.sync.dma_start(yr[:], R[:])

    # Pre-place activation table loads using a view of the tables where only
    # the combined natural_log_exp set provides Exp/Ln. Set ids stay aligned
    # with act_info.json (index order unchanged); this just stops the
    # first-match chooser from alternating between the exp-only and ln-only
    # sets (7 reloads x 1.28us -> 1 load). compile()'s own pass then sees
    # every activation covered and inserts nothing.
    import bass_rust as _bass_rust
    from concourse.hw_specs import get_activation_tables
    tabs = []
    for name, fns in get_activation_tables(nc.m.arch).items():
        if name != "natural_log_exp_and_others":
            fns = fns - {mybir.ActivationFunctionType.Exp,
                         mybir.ActivationFunctionType.Ln}
        tabs.append((name, fns))
    _bass_rust.insert_act_table_loads(nc, tabs)

    nc.compile()
    _CACHE[key] = nc
    return nc


def _get_runner(reps=1, outer=1):
    """Cached jitted SPMD executor (replicates bass2jax.run_bass_via_pjrt but
    reuses the jitted callable across calls so we don't re-trace every time)."""
    key = ("run", reps, outer)
    if key in _CACHE:
        return _CACHE[key]

    import jax
    from jax.sharding import Mesh, PartitionSpec
    try:
        from jax.experimental.shard_map import shard_map
    except ImportError:  # newer jax
        from jax.shard_map import shard_map
    from concourse import bass2jax, mybir

    nc = _build_program(reps, outer)
    bass2jax.install_neuronx_cc_hook()

    partition_name = nc.partition_id_tensor.name if nc.partition_id_tensor else None
    in_names, out_names, out_avals, zero_outs = [], [], [], []
    for alloc in nc.m.functions[0].allocations:
        if not isinstance(alloc, mybir.MemoryLocationSet):
            continue
        name = alloc.memorylocations[0].name
        if alloc.kind == "ExternalInput":
            if name != partition_name:
                in_names.append(name)
        elif alloc.kind == "ExternalOutput":
            shape = tuple(alloc.tensor_shape)
            dtype = mybir.dt.np(alloc.dtype)
            out_names.append(name)
            out_avals.append(jax.core.ShapedArray(shape, dtype))
            zero_outs.append(np.zeros(shape, dtype))
    n_params = len(in_names)
    n_outs = len(out_avals)
    all_in_names = list(in_names) + list(out_names)
    if partition_name is not None:
        all_in_names.append(partition_name)
    donate = tuple(range(n_params, n_params + n_outs))

    def _body(*args):
        operands = list(args)
        if partition_name is not None:
            operands.append(bass2jax.partition_id_tensor())
        outs = bass2jax._bass_exec_p.bind(
            *operands,
            out_avals=tuple(out_avals),
            in_names=tuple(all_in_names),
            out_names=tuple(out_names),
            lowering_input_output_aliases=(),
            sim_require_finite=True,
            sim_require_nnan=True,
            nc=nc,
        )
        return tuple(outs)

    devices = jax.devices()[:NCORES]
    mesh = Mesh(np.asarray(devices), ("core",))
    sharded = jax.jit(
        shard_map(_body, mesh=mesh,
                  in_specs=(PartitionSpec("core"),) * (n_params + n_outs),
                  out_specs=(PartitionSpec("core"),) * n_outs,
                  check_rep=False),
        donate_argnums=donate,
        keep_unused=True,
    )

    def run(in_maps):
        concat_in = [
            np.concatenate([np.asarray(m[name]) for m in in_maps], axis=0)
            for name in in_names
        ]
        concat_zeros = [
            np.zeros((NCORES * z.shape[0], *z.shape[1:]), z.dtype)
            for z in zero_outs
        ]
        out_arrs = sharded(*concat_in, *concat_zeros)
        return [
            {name: np.asarray(out_arrs[i]).reshape(NCORES, *out_avals[i].shape)[c]
             for i, name in enumerate(out_names)}
            for c in range(NCORES)
        ]

    _CACHE[key] = run
    return run


def _make_in_maps(x, k1, k2, bias):
    # host-side layout prep (sharding + padding + row packing only)
    kkf = np.concatenate(
        [k1.reshape(P, O), k2.reshape(P, O)], axis=1).astype(np.float32)
    kkp = np.full((384, G), -100.0, dtype=np.float32)
    for j in range(3):
        for a in range(3):
            srow = (FW * a + j) * C
            kkp[128 * j + 32 * a:128 * j + 32 * (a + 1)] = kkf[srow:srow + C]
    biasd = np.ascontiguousarray(
        np.tile(bias[None, :], (128, 1)).astype(np.float32))
    in_maps = []
    for b in range(NCORES):
        xp = np.full((C, SP), SHIFT, dtype=np.float32)
        xp[:, :H * W] = x[b].reshape(C, H * W)
        in_maps.append({"xp": xp, "kkp": kkp, "biasd": biasd})
    return in_maps


def kernel(x, k1, k2, bias, reps=1, outer=1):
    x = np.asarray(x, dtype=np.float32)
    k1 = np.asarray(k1, dtype=np.float32)
    k2 = np.asarray(k2, dtype=np.float32)
    bias = np.asarray(bias, dtype=np.float32)

    run = _get_runner(reps, outer)
    results = run(_make_in_maps(x, k1, k2, bias))
    out = np.empty((B, O, HO, WO), dtype=np.float32)
    for b in range(NCORES):
        yrb = results[b]["yr"]  # [128, NQ, O]; partition = 32*h2 + w
        full = yrb.reshape(4, 32, NQ, O).transpose(3, 2, 0, 1).reshape(
            O, 4 * NQ, 32)  # [o, h = 4q + h2, w]
        out[b] = full[:, :HO, :WO]
    return out
